# revision 29
# baseline (speedup 1.0000x reference)
"""BitNet FFN Trainium2 kernel (8-core SPMD).

Math (forward values of the STE reference):
  wq(w)  = clip(round(w/s), -1, 1) * s,  s = mean(|w|) + EPS        (ternary)
  xq(x)  = round(x/sx) * sx,  sx = max(absmax_row(x), EPS)/127      (int8 range)
  gate = sigmoid(xq @ wq_g.T); up = xq @ wq_u.T; h = gate*up
  out  = hq(h) @ wq_d.T

Design (v2):
  - Data-parallel over tokens (1024 tok/core), but weight ternarization is
    SHARDED: each core ternarizes 1/8 of each weight matrix (host passes the
    shard pre-transposed into matmul layout), then two AllGathers distribute
    the ternary bf16/fp16 weights to all cores.  This removes the 8x
    redundant weight DMA + tanh/round work the v1 kernel did.
  - Phase 1 computes G/U with the TERNARY WEIGHT as the stationary operand,
    so h' lands in PSUM already transposed ([ff, tok]).  h' = sigmoid(G*sxg)
    * U is written to a resident SBUF tile in fp16 -- no DRAM round trip.
  - Quantization of h' to int8-valued fp16 happens IN PLACE once the global
    per-token absmax is known; phase-3 matmuls chase the quantizer per
    k-block so the PE bubble at the boundary is tiny.
  - All matmuls run on exact integers (|int|<=127 activations, ternary
    weights, fp32 PSUM accumulation), scales are folded in fp32 outside.
    The only approximation is storing h' in fp16 before quantization.
"""

import sys

sys.path.insert(0, "/opt/trn_rl_repo")

import contextlib

import numpy as np

import concourse.tile as tile
from concourse import bacc, mybir
from concourse.masks import make_identity

F32 = mybir.dt.float32
BF16 = mybir.dt.bfloat16
FP16 = mybir.dt.float16
ADD = mybir.AluOpType.add
SUB = mybir.AluOpType.subtract
MULT = mybir.AluOpType.mult
MAX = mybir.AluOpType.max
BYP = mybir.AluOpType.bypass
AXX = mybir.AxisListType.X
AFT = mybir.ActivationFunctionType

EPS = 1e-5
CR = 12582912.0  # 1.5*2^23: fp32 RNE round-to-integer magic constant
ALPHA = 1.0986122886681098  # atanh(0.5)/0.5 : tanh(ALPHA*0.5) == 0.5
P = 128


def build_program(T, DM, FF, ncores):
    """Per-core SPMD program. T tokens/core; full DM/FF; ff sharded /ncores."""
    KD = DM // P           # d_model contraction blocks
    FB = FF // P           # ff 128-blocks
    MT = T // P            # token tiles
    SH = FF // ncores      # ff rows per shard
    SB = SH // P           # ff blocks per shard
    TN = min(512, T)       # token chunk per matmul
    NTC = T // TN          # token chunks
    CW = P                 # ff columns per phase-1 weight fetch
    MD = DM // P           # output dm blocks
    MQ = 4 if MD % 4 == 0 else 1   # dm blocks per phase-3 quad
    NW = float(FF * DM)
    assert T % P == 0 and DM % P == 0 and FF % (P * ncores) == 0
    assert FB % 4 == 0

    nc = bacc.Bacc(
        "TRN2",
        target_bir_lowering=False,
        debug=False,
        enable_asserts=False,
        num_devices=ncores,
    )

    x_d = nc.dram_tensor("x", [T, DM], F32, kind="ExternalInput")
    wgt_d = nc.dram_tensor("wgt_sh", [DM, SH], F32, kind="ExternalInput")
    wut_d = nc.dram_tensor("wut_sh", [DM, SH], F32, kind="ExternalInput")
    wdt_d = nc.dram_tensor("wdt_sh", [SH, DM], F32, kind="ExternalInput")
    out_d = nc.dram_tensor("out_t", [DM, T], F32, kind="ExternalOutput")

    RG = [list(range(ncores))]

    with tile.TileContext(nc, num_cores=ncores) as tc:
        with contextlib.ExitStack() as S:
            dram = S.enter_context(tc.tile_pool(name="dram", bufs=1, space="DRAM"))
            psum = S.enter_context(tc.tile_pool(name="psum", bufs=8, space="PSUM"))
            tiny = S.enter_context(tc.tile_pool(name="tiny", bufs=1))

            # DRAM scratch: AllGather bounce buffers.  wg/wu ternary weights
            # are gathered in NP ff-column pieces so the AllGathers pipeline
            # behind phase-1 compute.
            NP = 4 if SH % (4 * CW) == 0 else 1
            PW = SH // NP
            gin_p = [dram.tile([2 * DM, PW], BF16, name=f"ginp{i}") for i in range(NP)]
            gout_p = [
                dram.tile([ncores * 2 * DM, PW], BF16, addr_space="Shared",
                          name=f"goutp{i}")
                for i in range(NP)
            ]
            gin_d = dram.tile([SH, DM], FP16)
            gout_d = dram.tile([FF, DM], FP16, addr_space="Shared")
            ar_in = dram.tile([1, 4], F32)
            ar_out = dram.tile([1, 4], F32)
            rows_d = dram.tile([2, T], F32)   # rowify bounce (sx, amax)

            # persistent small tiles
            ones_row = tiny.tile([1, P], F32)
            nc.vector.memset(ones_row, 1.0)
            ones_col = tiny.tile([P, 1], F32)
            nc.vector.memset(ones_col, 1.0)
            ident = tiny.tile([P, P], F32)
            make_identity(nc, ident)

            sw_cells = tiny.tile([1, 3], F32)    # s_wg, s_wu, s_wd
            rsw_bc = tiny.tile([P, 3], F32)      # 1/s_w per partition
            sxu_row = tiny.tile([1, T], F32)     # sx*s_wu (survives to phase 2)
            sx_cols = tiny.tile([P, MT], F32)
            am_cols = tiny.tile([P, MT], F32)

            def rowify(cols, row, slot):
                """cols [P, MT] (col m = tokens m*P..(m+1)*P) -> row [1, T]."""
                pst = psum.tile([P, 512], F32, name="ps")
                nc.tensor.transpose(pst[:MT, :P], cols[:, :MT], ident)
                sb_t = tiny.tile([MT, P], F32, name="rowify_t")
                nc.vector.tensor_copy(sb_t, pst[:MT, :P])
                nc.sync.dma_start(rows_d[slot, :], sb_t[:, :])
                nc.sync.dma_start(row, rows_d[slot : slot + 1, :])

            def bcast(row, out_bc):
                """row [1, T] -> out_bc [P, T] (same value down partitions)."""
                for t in range(NTC):
                    psb = psum.tile([P, 512], F32, name="ps")
                    nc.tensor.matmul(
                        psb[:, :TN], ones_row, row[:, t * TN : (t + 1) * TN],
                        start=True, stop=True,
                    )
                    nc.vector.tensor_copy(out_bc[:, t * TN : (t + 1) * TN], psb[:, :TN])

            # ============ S0: sharded weight-scale scan + tiny AllReduce ======
            with contextlib.ExitStack() as pre:
                whold_p = pre.enter_context(tc.tile_pool(name="whold", bufs=1))
                wg_hold = whold_p.tile([P, KD, SH], F32)
                wu_hold = whold_p.tile([P, KD, SH], F32)
                nc.sync.dma_start(
                    wg_hold, wgt_d[:, :].rearrange("(i p) f -> p i f", p=P)
                )
                nc.scalar.dma_start(
                    wu_hold, wut_d[:, :].rearrange("(i p) f -> p i f", p=P)
                )
                acc = tiny.tile([P, 3], F32)
                nc.vector.tensor_reduce(
                    acc[:, 0:1], wg_hold.rearrange("p i f -> p (i f)"),
                    axis=AXX, op=ADD, apply_absolute_value=True,
                )
                nc.vector.tensor_reduce(
                    acc[:, 1:2], wu_hold.rearrange("p i f -> p (i f)"),
                    axis=AXX, op=ADD, apply_absolute_value=True,
                )
                # wd scan: streaming (tiles are reloaded later for ternarize)
                with tc.tile_pool(name="wdscan", bufs=3) as wds_p:
                    nc.vector.memset(acc[:, 2:3], 0.0)
                    for i in range(SB):
                        wdt = wds_p.tile([P, DM], F32, name="wdscan_t")
                        nc.scalar.dma_start(wdt, wdt_d[i * P : (i + 1) * P, :])
                        part = wds_p.tile([P, 1], F32, name="wdscan_s")
                        nc.vector.tensor_reduce(
                            part, wdt, axis=AXX, op=ADD, apply_absolute_value=True
                        )
                        nc.vector.tensor_tensor(
                            out=acc[:, 2:3], in0=acc[:, 2:3], in1=part, op=ADD
                        )

                pss = psum.tile([P, 512], F32, name="ps")
                nc.tensor.matmul(pss[:3, :1], acc[:, :3], ones_col, start=True, stop=True)
                sums_sb = tiny.tile([3, 1], F32)
                nc.vector.tensor_copy(sums_sb, pss[:3, :1])
                nc.sync.dma_start(ar_in[0, :3], sums_sb[:, 0])
                nc.gpsimd.collective_compute(
                    "AllReduce",
                    ADD,
                    replica_groups=RG,
                    ins=[ar_in[:, :3].opt()],
                    outs=[ar_out[:, :3].opt()],
                )
                arld = tiny.tile([1, 3], F32)
                nc.sync.dma_start(arld, ar_out[:, :3])
                nc.vector.tensor_scalar(
                    out=sw_cells, in0=arld, scalar1=1.0 / NW, scalar2=EPS,
                    op0=MULT, op1=ADD,
                )
                binv = tiny.tile([1, 3], F32)
                nc.vector.reciprocal(binv, sw_cells)
                psb3 = psum.tile([P, 512], F32, name="ps")
                nc.tensor.matmul(psb3[:, :3], ones_row, binv, start=True, stop=True)
                nc.vector.tensor_copy(rsw_bc, psb3[:, :3])

                # ---- ternarize wg/wu shards piece-major (DVE round+clip),
                #      kick each piece's AllGather as soon as it is bounced.
                # tq is deep-buffered so the DVE ladder is not starved by the
                # per-DMA completion latency of the bounce writes; the bounce
                # DMAs ride the scalar ring so they never block the sync-ring
                # phase-1 weight fetches.
                with tc.tile_pool(name="tern", bufs=3) as tern_p, tc.tile_pool(
                    name="ternq", bufs=10
                ) as ternq_p:
                    for ch in range(NP):
                        fs = slice(ch * PW, (ch + 1) * PW)
                        for src_hold, base, col in (
                            (wg_hold, 0, 0),
                            (wu_hold, DM, 1),
                        ):
                            for i in range(KD):
                                t1 = tern_p.tile([P, PW], F32, name="tern_1")
                                nc.vector.tensor_scalar(
                                    out=t1, in0=src_hold[:, i, fs],
                                    scalar1=rsw_bc[:, col : col + 1], scalar2=1.25,
                                    op0=MULT, op1=mybir.AluOpType.min,
                                )
                                t2 = tern_p.tile([P, PW], F32, name="tern_2")
                                nc.vector.tensor_scalar(
                                    out=t2, in0=t1, scalar1=-1.25, scalar2=CR,
                                    op0=MAX, op1=ADD,
                                )
                                tq = ternq_p.tile([P, PW], BF16, name="tern_q")
                                nc.vector.tensor_scalar(
                                    out=tq, in0=t2, scalar1=CR, scalar2=None,
                                    op0=SUB, op1=BYP,
                                )
                                nc.scalar.dma_start(
                                    gin_p[ch][base + i * P : base + (i + 1) * P, :],
                                    tq,
                                )
                        nc.gpsimd.collective_compute(
                            "AllGather",
                            BYP,
                            replica_groups=RG,
                            ins=[gin_p[ch][:].opt()],
                            outs=[gout_p[ch][:].opt()],
                        )

            # ============ persistent big tiles across phases 1..3 =============
            big_p = S.enter_context(tc.tile_pool(name="big", bufs=1))
            hpT = big_p.tile([P, FB, T], FP16)    # h' (later: quantized h)
            mxa = big_p.tile([P, T], F32)         # running absmax of |h'|
            nc.vector.memset(mxa, 0.0)

            with contextlib.ExitStack() as mid:
                mid_p = mid.enter_context(tc.tile_pool(name="mid", bufs=1))
                xqt = mid_p.tile([P, KD, T], BF16)   # x quantized, transposed
                sxg_bc = mid_p.tile([P, T], FP16)
                sx_row = mid_p.tile([1, T], F32)
                sxg_row = mid_p.tile([1, T], F32)

                # ======== phase 0: x quantization (overlaps the AllGather) ====
                with tc.tile_pool(name="xw", bufs=1) as xw_p:
                    for m in range(MT):
                        xt = xw_p.tile([P, DM], F32, name="xt")
                        nc.sync.dma_start(xt, x_d[m * P : (m + 1) * P, :])
                        amax = xw_p.tile([P, 1], F32, name="amax")
                        nc.vector.tensor_reduce(
                            amax, xt, axis=AXX, op=MAX, apply_absolute_value=True
                        )
                        nc.vector.tensor_scalar(
                            out=sx_cols[:, m : m + 1], in0=amax, scalar1=EPS,
                            scalar2=1.0 / 127.0, op0=MAX, op1=MULT,
                        )
                        rx = xw_p.tile([P, 1], F32, name="rx")
                        nc.vector.reciprocal(rx, sx_cols[:, m : m + 1])
                        nc.vector.tensor_scalar(
                            out=xt, in0=xt, scalar1=rx, scalar2=CR, op0=MULT, op1=ADD
                        )
                        xq = xw_p.tile([P, DM], BF16, name="xq")
                        nc.vector.tensor_scalar(
                            out=xq, in0=xt, scalar1=CR, scalar2=None, op0=SUB, op1=BYP
                        )
                        nc.sync.dma_start_transpose(
                            xqt[:, :, m * P : (m + 1) * P], xq
                        )
                    rowify(sx_cols, sx_row, 0)
                    nc.vector.tensor_scalar(
                        out=sxg_row, in0=sx_row, scalar1=sw_cells[:, 0:1],
                        scalar2=None, op0=MULT, op1=BYP,
                    )
                    nc.vector.tensor_scalar(
                        out=sxu_row, in0=sx_row, scalar1=sw_cells[:, 1:2],
                        scalar2=None, op0=MULT, op1=BYP,
                    )
                    bcast(sxg_row, sxg_bc)

                # ---- ternarize wd shard (reload) -> gin_d -> AG (late) -------
                with tc.tile_pool(name="ternd", bufs=1) as td_p:
                    for i in range(SB):
                        wdt = td_p.tile([P, DM], F32, name="ternd_in")
                        nc.scalar.dma_start(wdt, wdt_d[i * P : (i + 1) * P, :])
                        nc.vector.tensor_scalar(
                            out=wdt, in0=wdt, scalar1=rsw_bc[:, 2:3], scalar2=CR,
                            op0=MULT, op1=ADD,
                        )
                        nc.vector.tensor_scalar(
                            out=wdt, in0=wdt, scalar1=CR, scalar2=1.0,
                            op0=SUB, op1=mybir.AluOpType.min,
                        )
                        tq = td_p.tile([P, DM], FP16, name="ternd_q")
                        nc.vector.tensor_scalar(
                            out=tq, in0=wdt, scalar1=-1.0, scalar2=None,
                            op0=MAX, op1=BYP,
                        )
                        nc.scalar.dma_start(gin_d[i * P : (i + 1) * P, :], tq)
                nc.gpsimd.collective_compute(
                    "AllGather",
                    BYP,
                    replica_groups=RG,
                    ins=[gin_d[:].opt()],
                    outs=[gout_d[:].opt()],
                )

                # ======== phase 1: gate/up matmuls, h' -> hpT (fp16) ==========
                # piece-outer so each block only waits on its piece's AG
                with contextlib.ExitStack() as ph1:
                    wch_p = ph1.enter_context(tc.tile_pool(name="wch", bufs=2))
                    gt_p = ph1.enter_context(tc.tile_pool(name="gt", bufs=1))
                    for ch in range(NP):
                        for c in range(ncores):
                            for sub in range(PW // CW):
                                # one fetch holds the G and U k-stacks for
                                # this 128-wide ff block: [P, 2*KD, CW]
                                guch = wch_p.tile(
                                    [P, 2 * KD, CW], BF16, name="guch"
                                )
                                base = c * 2 * DM
                                cs = slice(sub * CW, (sub + 1) * CW)
                                nc.sync.dma_start(
                                    guch,
                                    gout_p[ch][
                                        base : base + 2 * DM, cs
                                    ].rearrange("(i p) f -> p i f", p=P),
                                )
                                f = c * SB + ch * (PW // P) + sub
                                psG = [
                                    psum.tile([P, 512], F32, name="ps")
                                    for _ in range(NTC)
                                ]
                                psU = [
                                    psum.tile([P, 512], F32, name="ps")
                                    for _ in range(NTC)
                                ]
                                for ps_list, koff in ((psG, 0), (psU, KD)):
                                    for k in range(KD):
                                        lhsT = guch[:, koff + k, :]
                                        st, sp = (k == 0), (k == KD - 1)
                                        for t in range(NTC):
                                            nc.tensor.matmul(
                                                ps_list[t][:, :TN],
                                                lhsT,
                                                xqt[:, k, t * TN : (t + 1) * TN],
                                                start=st,
                                                stop=sp,
                                            )
                                gt = gt_p.tile([P, T], F32, name="gt")
                                for t in range(NTC):
                                    ts_ = slice(t * TN, (t + 1) * TN)
                                    nc.vector.tensor_tensor(
                                        out=gt[:, ts_], in0=psG[t][:, :TN],
                                        in1=sxg_bc[:, ts_], op=MULT,
                                    )
                                nc.scalar.activation(
                                    out=gt, in_=gt, func=AFT.Sigmoid
                                )
                                for t in range(NTC):
                                    ts_ = slice(t * TN, (t + 1) * TN)
                                    nc.vector.tensor_tensor(
                                        out=hpT[:, f, ts_], in0=gt[:, ts_],
                                        in1=psU[t][:, :TN], op=MULT,
                                    )
                                # gt is dead after the h' write: reuse it as
                                # the |h'| scratch for the running absmax
                                nc.scalar.activation(
                                    out=gt, in_=hpT[:, f, :], func=AFT.Abs
                                )
                                nc.vector.tensor_tensor(
                                    out=mxa, in0=mxa, in1=gt, op=MAX
                                )

            # ============ phase 2: global h scales + in-place quantize ========
            with contextlib.ExitStack() as ph2:
                ph2_p = ph2.enter_context(tc.tile_pool(name="ph2", bufs=1))
                # per-token absmax over partitions: PE transpose + free-axis max
                for m in range(MT):
                    pst = psum.tile([P, 512], F32, name="ps")
                    nc.tensor.transpose(
                        pst[:P, :P], mxa[:, m * P : (m + 1) * P], ident
                    )
                    nc.vector.tensor_reduce(
                        am_cols[:, m : m + 1], pst[:P, :P], axis=AXX, op=MAX
                    )
                amax_row = ph2_p.tile([1, T], F32)
                sh_row = ph2_p.tile([1, T], F32)
                rph_row = ph2_p.tile([1, T], F32)
                shd_row = ph2_p.tile([1, T], F32)
                rowify(am_cols, amax_row, 1)
                nc.vector.tensor_tensor(
                    out=amax_row, in0=amax_row, in1=sxu_row, op=MULT
                )
                nc.vector.tensor_scalar(
                    out=sh_row, in0=amax_row, scalar1=EPS, scalar2=1.0 / 127.0,
                    op0=MAX, op1=MULT,
                )
                nc.vector.reciprocal(rph_row, sh_row)
                nc.vector.tensor_tensor(
                    out=rph_row, in0=rph_row, in1=sxu_row, op=MULT
                )
                nc.vector.tensor_scalar(
                    out=shd_row, in0=sh_row, scalar1=sw_cells[:, 2:3],
                    scalar2=None, op0=MULT, op1=BYP,
                )
                rph_bc = big_p.tile([P, T], F32)
                shd_bc = big_p.tile([P, T], F32)
                bcast(rph_row, rph_bc)
                bcast(shd_row, shd_bc)
                qtmp = ph2_p.tile([P, T], F32)
                for k in range(FB):
                    nc.vector.tensor_tensor(
                        out=qtmp, in0=hpT[:, k, :], in1=rph_bc, op=MULT
                    )
                    nc.vector.tensor_scalar(
                        out=hpT[:, k, :], in0=qtmp, scalar1=CR, scalar2=CR,
                        op0=ADD, op1=SUB,
                    )

                # ======== phase 3: down projection ============================
                with contextlib.ExitStack() as ph3:
                    wd3_p = ph3.enter_context(tc.tile_pool(name="wd3", bufs=3))
                    fin_p = ph3.enter_context(tc.tile_pool(name="fin", bufs=2))
                    DMQ = MQ * P
                    for q in range(MD // MQ):
                        ps3 = [
                            psum.tile([P, 512], F32, name="ps")
                            for _ in range(MQ * NTC)
                        ]
                        for g in range(FB // 4):
                            wtile = wd3_p.tile([P, 4, DMQ], FP16, name="wd3t")
                            nc.sync.dma_start(
                                wtile,
                                gout_d[
                                    g * 4 * P : (g + 1) * 4 * P,
                                    q * DMQ : (q + 1) * DMQ,
                                ].rearrange("(i p) f -> p i f", p=P),
                            )
                            for kb in range(4):
                                k = g * 4 + kb
                                st, sp = (k == 0), (k == FB - 1)
                                for mi in range(MQ):
                                    lhsT = wtile[:, kb, mi * P : (mi + 1) * P]
                                    for t in range(NTC):
                                        nc.tensor.matmul(
                                            ps3[mi * NTC + t][:, :TN],
                                            lhsT,
                                            hpT[:, k, t * TN : (t + 1) * TN],
                                            start=st,
                                            stop=sp,
                                        )
                        for mi in range(MQ):
                            md = q * MQ + mi
                            ot = fin_p.tile([P, T], F32, name="ot")
                            for t in range(NTC):
                                ts_ = slice(t * TN, (t + 1) * TN)
                                nc.vector.tensor_tensor(
                                    out=ot[:, ts_], in0=ps3[mi * NTC + t][:, :TN],
                                    in1=shd_bc[:, ts_], op=MULT,
                                )
                            nc.scalar.dma_start(
                                out_d[md * P : (md + 1) * P, :], ot
                            )

    nc.compile()
    return nc


_CACHE = {}
TRACE = False
LAST_RESULTS = None


def _get_program(T, DM, FF, ncores):
    key = (T, DM, FF, ncores)
    if key not in _CACHE:
        _CACHE[key] = build_program(T, DM, FF, ncores)
    return _CACHE[key]


def kernel(x, w_gate, w_up, w_down):
    from concourse.bass_utils import run_bass_kernel_spmd

    x = np.asarray(x, dtype=np.float32)
    w_gate = np.asarray(w_gate, dtype=np.float32)
    w_up = np.asarray(w_up, dtype=np.float32)
    w_down = np.asarray(w_down, dtype=np.float32)

    B, S, DM = x.shape
    FF = w_gate.shape[0]
    NCORES = 8
    NTOK = B * S
    T = NTOK // NCORES
    SH = FF // NCORES

    xf = np.ascontiguousarray(x.reshape(NTOK, DM))
    nc = _get_program(T, DM, FF, NCORES)

    in_maps = []
    for c in range(NCORES):
        sl = slice(c * SH, (c + 1) * SH)
        in_maps.append(
            {
                "x": np.ascontiguousarray(xf[c * T : (c + 1) * T]),
                "wgt_sh": np.ascontiguousarray(w_gate[sl].T),
                "wut_sh": np.ascontiguousarray(w_up[sl].T),
                "wdt_sh": np.ascontiguousarray(w_down[:, sl].T),
            }
        )

    res = run_bass_kernel_spmd(
        nc, in_maps, core_ids=list(range(NCORES)), trace=TRACE
    )
    global LAST_RESULTS
    LAST_RESULTS = res
    out = np.empty((NTOK, DM), dtype=np.float32)
    for c in range(NCORES):
        out[c * T : (c + 1) * T] = res.results[c]["out_t"].T
    return out.reshape(B, S, DM)


# revision 33
# speedup vs baseline: 1.0009x; 1.0009x over previous
"""BitNet FFN Trainium2 kernel (8-core SPMD).

Math (forward values of the STE reference):
  wq(w)  = clip(round(w/s), -1, 1) * s,  s = mean(|w|) + EPS        (ternary)
  xq(x)  = round(x/sx) * sx,  sx = max(absmax_row(x), EPS)/127      (int8 range)
  gate = sigmoid(xq @ wq_g.T); up = xq @ wq_u.T; h = gate*up
  out  = hq(h) @ wq_d.T

Design (v2):
  - Data-parallel over tokens (1024 tok/core), but weight ternarization is
    SHARDED: each core ternarizes 1/8 of each weight matrix (host passes the
    shard pre-transposed into matmul layout), then two AllGathers distribute
    the ternary bf16/fp16 weights to all cores.  This removes the 8x
    redundant weight DMA + tanh/round work the v1 kernel did.
  - Phase 1 computes G/U with the TERNARY WEIGHT as the stationary operand,
    so h' lands in PSUM already transposed ([ff, tok]).  h' = sigmoid(G*sxg)
    * U is written to a resident SBUF tile in fp16 -- no DRAM round trip.
  - Quantization of h' to int8-valued fp16 happens IN PLACE once the global
    per-token absmax is known; phase-3 matmuls chase the quantizer per
    k-block so the PE bubble at the boundary is tiny.
  - All matmuls run on exact integers (|int|<=127 activations, ternary
    weights, fp32 PSUM accumulation), scales are folded in fp32 outside.
    The only approximation is storing h' in fp16 before quantization.
"""

import sys

sys.path.insert(0, "/opt/trn_rl_repo")

import contextlib

import numpy as np

import concourse.tile as tile
from concourse import bacc, mybir
from concourse.masks import make_identity

F32 = mybir.dt.float32
BF16 = mybir.dt.bfloat16
FP16 = mybir.dt.float16
ADD = mybir.AluOpType.add
SUB = mybir.AluOpType.subtract
MULT = mybir.AluOpType.mult
MAX = mybir.AluOpType.max
BYP = mybir.AluOpType.bypass
AXX = mybir.AxisListType.X
AFT = mybir.ActivationFunctionType

EPS = 1e-5
CR = 12582912.0  # 1.5*2^23: fp32 RNE round-to-integer magic constant
ALPHA = 1.0986122886681098  # atanh(0.5)/0.5 : tanh(ALPHA*0.5) == 0.5
P = 128


def build_program(T, DM, FF, ncores):
    """Per-core SPMD program. T tokens/core; full DM/FF; ff sharded /ncores."""
    KD = DM // P           # d_model contraction blocks
    FB = FF // P           # ff 128-blocks
    MT = T // P            # token tiles
    SH = FF // ncores      # ff rows per shard
    SB = SH // P           # ff blocks per shard
    TN = min(512, T)       # token chunk per matmul
    NTC = T // TN          # token chunks
    CW = P                 # ff columns per phase-1 weight fetch
    MD = DM // P           # output dm blocks
    MQ = 4 if MD % 4 == 0 else 1   # dm blocks per phase-3 quad
    NW = float(FF * DM)
    assert T % P == 0 and DM % P == 0 and FF % (P * ncores) == 0
    assert FB % 4 == 0

    nc = bacc.Bacc(
        "TRN2",
        target_bir_lowering=False,
        debug=False,
        enable_asserts=False,
        num_devices=ncores,
    )

    x_d = nc.dram_tensor("x", [T, DM], F32, kind="ExternalInput")
    wgt_d = nc.dram_tensor("wgt_sh", [DM, SH], F32, kind="ExternalInput")
    wut_d = nc.dram_tensor("wut_sh", [DM, SH], F32, kind="ExternalInput")
    wdt_d = nc.dram_tensor("wdt_sh", [SH, DM], F32, kind="ExternalInput")
    out_d = nc.dram_tensor("out_t", [DM, T], F32, kind="ExternalOutput")

    RG = [list(range(ncores))]

    with tile.TileContext(nc, num_cores=ncores) as tc:
        with contextlib.ExitStack() as S:
            dram = S.enter_context(tc.tile_pool(name="dram", bufs=1, space="DRAM"))
            psum = S.enter_context(tc.tile_pool(name="psum", bufs=8, space="PSUM"))
            tiny = S.enter_context(tc.tile_pool(name="tiny", bufs=1))

            # DRAM scratch: AllGather bounce buffers.  wg/wu ternary weights
            # are gathered in NP ff-column pieces so the AllGathers pipeline
            # behind phase-1 compute.
            NP = 4 if SH % (4 * CW) == 0 else 1
            PW = SH // NP
            gin_p = [dram.tile([2 * DM, PW], BF16, name=f"ginp{i}") for i in range(NP)]
            gout_p = [
                dram.tile([ncores * 2 * DM, PW], BF16, addr_space="Shared",
                          name=f"goutp{i}")
                for i in range(NP)
            ]
            gin_d = dram.tile([SH, DM], FP16)
            gout_d = dram.tile([FF, DM], FP16, addr_space="Shared")
            ar_in = dram.tile([1, 4], F32)
            ar_out = dram.tile([1, 4], F32)
            rows_d = dram.tile([2, T], F32)   # rowify bounce (sx, amax)

            # persistent small tiles
            ones_row = tiny.tile([1, P], F32)
            nc.vector.memset(ones_row, 1.0)
            ones_col = tiny.tile([P, 1], F32)
            nc.vector.memset(ones_col, 1.0)
            ident = tiny.tile([P, P], F32)
            make_identity(nc, ident)

            sw_cells = tiny.tile([1, 3], F32)    # s_wg, s_wu, s_wd
            rsw_bc = tiny.tile([P, 3], F32)      # 1/s_w per partition
            sxu_row = tiny.tile([1, T], F32)     # sx*s_wu (survives to phase 2)
            sx_cols = tiny.tile([P, MT], F32)
            am_cols = tiny.tile([P, MT], F32)

            def rowify(cols, row, slot):
                """cols [P, MT] (col m = tokens m*P..(m+1)*P) -> row [1, T]."""
                pst = psum.tile([P, 512], F32, name="ps")
                nc.tensor.transpose(pst[:MT, :P], cols[:, :MT], ident)
                sb_t = tiny.tile([MT, P], F32, name="rowify_t")
                nc.vector.tensor_copy(sb_t, pst[:MT, :P])
                nc.sync.dma_start(rows_d[slot, :], sb_t[:, :])
                nc.sync.dma_start(row, rows_d[slot : slot + 1, :])

            def bcast(row, out_bc):
                """row [1, T] -> out_bc [P, T] (same value down partitions)."""
                for t in range(NTC):
                    psb = psum.tile([P, 512], F32, name="ps")
                    nc.tensor.matmul(
                        psb[:, :TN], ones_row, row[:, t * TN : (t + 1) * TN],
                        start=True, stop=True,
                    )
                    nc.vector.tensor_copy(out_bc[:, t * TN : (t + 1) * TN], psb[:, :TN])

            # ============ S0: sharded weight-scale scan + tiny AllReduce ======
            with contextlib.ExitStack() as pre:
                whold_p = pre.enter_context(tc.tile_pool(name="whold", bufs=1))
                wg_hold = whold_p.tile([P, KD, SH], F32)
                wu_hold = whold_p.tile([P, KD, SH], F32)
                nc.sync.dma_start(
                    wg_hold, wgt_d[:, :].rearrange("(i p) f -> p i f", p=P)
                )
                nc.scalar.dma_start(
                    wu_hold, wut_d[:, :].rearrange("(i p) f -> p i f", p=P)
                )
                acc = tiny.tile([P, 3], F32)
                nc.vector.tensor_reduce(
                    acc[:, 0:1], wg_hold.rearrange("p i f -> p (i f)"),
                    axis=AXX, op=ADD, apply_absolute_value=True,
                )
                nc.vector.tensor_reduce(
                    acc[:, 1:2], wu_hold.rearrange("p i f -> p (i f)"),
                    axis=AXX, op=ADD, apply_absolute_value=True,
                )
                # wd scan: streaming (tiles are reloaded later for ternarize)
                with tc.tile_pool(name="wdscan", bufs=3) as wds_p:
                    nc.vector.memset(acc[:, 2:3], 0.0)
                    for i in range(SB):
                        wdt = wds_p.tile([P, DM], F32, name="wdscan_t")
                        nc.gpsimd.dma_start(wdt, wdt_d[i * P : (i + 1) * P, :])
                        part = wds_p.tile([P, 1], F32, name="wdscan_s")
                        nc.vector.tensor_reduce(
                            part, wdt, axis=AXX, op=ADD, apply_absolute_value=True
                        )
                        nc.vector.tensor_tensor(
                            out=acc[:, 2:3], in0=acc[:, 2:3], in1=part, op=ADD
                        )

                pss = psum.tile([P, 512], F32, name="ps")
                nc.tensor.matmul(pss[:3, :1], acc[:, :3], ones_col, start=True, stop=True)
                sums_sb = tiny.tile([3, 1], F32)
                nc.vector.tensor_copy(sums_sb, pss[:3, :1])
                nc.sync.dma_start(ar_in[0, :3], sums_sb[:, 0])
                nc.gpsimd.collective_compute(
                    "AllReduce",
                    ADD,
                    replica_groups=RG,
                    ins=[ar_in[:, :3].opt()],
                    outs=[ar_out[:, :3].opt()],
                )
                arld = tiny.tile([1, 3], F32)
                nc.sync.dma_start(arld, ar_out[:, :3])
                nc.vector.tensor_scalar(
                    out=sw_cells, in0=arld, scalar1=1.0 / NW, scalar2=EPS,
                    op0=MULT, op1=ADD,
                )
                binv = tiny.tile([1, 3], F32)
                nc.vector.reciprocal(binv, sw_cells)
                psb3 = psum.tile([P, 512], F32, name="ps")
                nc.tensor.matmul(psb3[:, :3], ones_row, binv, start=True, stop=True)
                nc.vector.tensor_copy(rsw_bc, psb3[:, :3])

                # ---- ternarize wg/wu shards piece-major (DVE round+clip),
                #      kick each piece's AllGather as soon as it is bounced.
                # tq is deep-buffered so the DVE ladder is not starved by the
                # per-DMA completion latency of the bounce writes; the bounce
                # DMAs ride the scalar ring so they never block the sync-ring
                # phase-1 weight fetches.
                with tc.tile_pool(name="tern", bufs=3) as tern_p, tc.tile_pool(
                    name="ternq", bufs=10
                ) as ternq_p:
                    for ch in range(NP):
                        fs = slice(ch * PW, (ch + 1) * PW)
                        for src_hold, base, col in (
                            (wg_hold, 0, 0),
                            (wu_hold, DM, 1),
                        ):
                            for i in range(KD):
                                t1 = tern_p.tile([P, PW], F32, name="tern_1")
                                nc.vector.tensor_scalar(
                                    out=t1, in0=src_hold[:, i, fs],
                                    scalar1=rsw_bc[:, col : col + 1], scalar2=1.25,
                                    op0=MULT, op1=mybir.AluOpType.min,
                                )
                                t2 = tern_p.tile([P, PW], F32, name="tern_2")
                                nc.vector.tensor_scalar(
                                    out=t2, in0=t1, scalar1=-1.25, scalar2=CR,
                                    op0=MAX, op1=ADD,
                                )
                                tq = ternq_p.tile([P, PW], BF16, name="tern_q")
                                nc.vector.tensor_scalar(
                                    out=tq, in0=t2, scalar1=CR, scalar2=None,
                                    op0=SUB, op1=BYP,
                                )
                                nc.scalar.dma_start(
                                    gin_p[ch][base + i * P : base + (i + 1) * P, :],
                                    tq,
                                )
                        nc.gpsimd.collective_compute(
                            "AllGather",
                            BYP,
                            replica_groups=RG,
                            ins=[gin_p[ch][:].opt()],
                            outs=[gout_p[ch][:].opt()],
                        )

            # ============ persistent big tiles across phases 1..3 =============
            big_p = S.enter_context(tc.tile_pool(name="big", bufs=1))
            hpT = big_p.tile([P, FB, T], FP16)    # h' (later: quantized h)
            mxa = big_p.tile([P, T], F32)         # running absmax of |h'|
            nc.vector.memset(mxa, 0.0)

            with contextlib.ExitStack() as mid:
                mid_p = mid.enter_context(tc.tile_pool(name="mid", bufs=1))
                xqt = mid_p.tile([P, KD, T], BF16)   # x quantized, transposed
                sxg_bc = mid_p.tile([P, T], FP16)

                # ======== phase 0: x quantization (overlaps the AllGather) ====
                with tc.tile_pool(name="xw", bufs=2) as xw_p:
                    for m in range(MT):
                        xt = xw_p.tile([P, DM], F32, name="xt")
                        nc.sync.dma_start(xt, x_d[m * P : (m + 1) * P, :])
                        amax = xw_p.tile([P, 1], F32, name="amax")
                        nc.vector.tensor_reduce(
                            amax, xt, axis=AXX, op=MAX, apply_absolute_value=True
                        )
                        nc.vector.tensor_scalar(
                            out=sx_cols[:, m : m + 1], in0=amax, scalar1=EPS,
                            scalar2=1.0 / 127.0, op0=MAX, op1=MULT,
                        )
                        rx = xw_p.tile([P, 1], F32, name="rx")
                        nc.vector.reciprocal(rx, sx_cols[:, m : m + 1])
                        nc.vector.tensor_scalar(
                            out=xt, in0=xt, scalar1=rx, scalar2=CR, op0=MULT, op1=ADD
                        )
                        xq = xw_p.tile([P, DM], BF16, name="xq")
                        nc.vector.tensor_scalar(
                            out=xq, in0=xt, scalar1=CR, scalar2=None, op0=SUB, op1=BYP
                        )
                        nc.sync.dma_start_transpose(
                            xqt[:, :, m * P : (m + 1) * P], xq
                        )
                with tc.tile_pool(name="ph0rows", bufs=1) as r0_p:
                    sx_row = r0_p.tile([1, T], F32, name="sx_row")
                    sxg_row = r0_p.tile([1, T], F32, name="sxg_row")
                    rowify(sx_cols, sx_row, 0)
                    nc.vector.tensor_scalar(
                        out=sxg_row, in0=sx_row, scalar1=sw_cells[:, 0:1],
                        scalar2=None, op0=MULT, op1=BYP,
                    )
                    nc.vector.tensor_scalar(
                        out=sxu_row, in0=sx_row, scalar1=sw_cells[:, 1:2],
                        scalar2=None, op0=MULT, op1=BYP,
                    )
                    bcast(sxg_row, sxg_bc)

                # ---- ternarize wd shard (reload) -> gin_d -> AG (late) -------
                with tc.tile_pool(name="ternd", bufs=1) as td_p:
                    for i in range(SB):
                        wdt = td_p.tile([P, DM], F32, name="ternd_in")
                        nc.scalar.dma_start(wdt, wdt_d[i * P : (i + 1) * P, :])
                        nc.vector.tensor_scalar(
                            out=wdt, in0=wdt, scalar1=rsw_bc[:, 2:3], scalar2=CR,
                            op0=MULT, op1=ADD,
                        )
                        nc.vector.tensor_scalar(
                            out=wdt, in0=wdt, scalar1=CR, scalar2=1.0,
                            op0=SUB, op1=mybir.AluOpType.min,
                        )
                        tq = td_p.tile([P, DM], FP16, name="ternd_q")
                        nc.vector.tensor_scalar(
                            out=tq, in0=wdt, scalar1=-1.0, scalar2=None,
                            op0=MAX, op1=BYP,
                        )
                        nc.scalar.dma_start(gin_d[i * P : (i + 1) * P, :], tq)
                nc.gpsimd.collective_compute(
                    "AllGather",
                    BYP,
                    replica_groups=RG,
                    ins=[gin_d[:].opt()],
                    outs=[gout_d[:].opt()],
                )

                # ======== phase 1: gate/up matmuls, h' -> hpT (fp16) ==========
                # piece-outer so each block only waits on its piece's AG
                with contextlib.ExitStack() as ph1:
                    wch_p = ph1.enter_context(tc.tile_pool(name="wch", bufs=2))
                    gt_p = ph1.enter_context(tc.tile_pool(name="gt", bufs=1))
                    for ch in range(NP):
                        for c in range(ncores):
                            for sub in range(PW // CW):
                                # one fetch holds the G and U k-stacks for
                                # this 128-wide ff block: [P, 2*KD, CW]
                                guch = wch_p.tile(
                                    [P, 2 * KD, CW], BF16, name="guch"
                                )
                                base = c * 2 * DM
                                cs = slice(sub * CW, (sub + 1) * CW)
                                nc.sync.dma_start(
                                    guch,
                                    gout_p[ch][
                                        base : base + 2 * DM, cs
                                    ].rearrange("(i p) f -> p i f", p=P),
                                )
                                f = c * SB + ch * (PW // P) + sub
                                psG = [
                                    psum.tile([P, 512], F32, name="ps")
                                    for _ in range(NTC)
                                ]
                                psU = [
                                    psum.tile([P, 512], F32, name="ps")
                                    for _ in range(NTC)
                                ]
                                for ps_list, koff in ((psG, 0), (psU, KD)):
                                    for k in range(KD):
                                        lhsT = guch[:, koff + k, :]
                                        st, sp = (k == 0), (k == KD - 1)
                                        for t in range(NTC):
                                            nc.tensor.matmul(
                                                ps_list[t][:, :TN],
                                                lhsT,
                                                xqt[:, k, t * TN : (t + 1) * TN],
                                                start=st,
                                                stop=sp,
                                            )
                                gt = gt_p.tile([P, T], F32, name="gt")
                                for t in range(NTC):
                                    ts_ = slice(t * TN, (t + 1) * TN)
                                    nc.vector.tensor_tensor(
                                        out=gt[:, ts_], in0=psG[t][:, :TN],
                                        in1=sxg_bc[:, ts_], op=MULT,
                                    )
                                nc.scalar.activation(
                                    out=gt, in_=gt, func=AFT.Sigmoid
                                )
                                for t in range(NTC):
                                    ts_ = slice(t * TN, (t + 1) * TN)
                                    nc.vector.tensor_tensor(
                                        out=hpT[:, f, ts_], in0=gt[:, ts_],
                                        in1=psU[t][:, :TN], op=MULT,
                                    )
                                # gt is dead after the h' write: reuse it as
                                # the |h'| scratch for the running absmax
                                nc.scalar.activation(
                                    out=gt, in_=hpT[:, f, :], func=AFT.Abs
                                )
                                nc.vector.tensor_tensor(
                                    out=mxa, in0=mxa, in1=gt, op=MAX
                                )

            # ============ phase 2: global h scales + in-place quantize ========
            with contextlib.ExitStack() as ph2:
                ph2_p = ph2.enter_context(tc.tile_pool(name="ph2", bufs=1))
                # per-token absmax over partitions: PE transpose + free-axis max
                for m in range(MT):
                    pst = psum.tile([P, 512], F32, name="ps")
                    nc.tensor.transpose(
                        pst[:P, :P], mxa[:, m * P : (m + 1) * P], ident
                    )
                    nc.vector.tensor_reduce(
                        am_cols[:, m : m + 1], pst[:P, :P], axis=AXX, op=MAX
                    )
                amax_row = ph2_p.tile([1, T], F32)
                sh_row = ph2_p.tile([1, T], F32)
                rph_row = ph2_p.tile([1, T], F32)
                shd_row = ph2_p.tile([1, T], F32)
                rowify(am_cols, amax_row, 1)
                nc.vector.tensor_tensor(
                    out=amax_row, in0=amax_row, in1=sxu_row, op=MULT
                )
                nc.vector.tensor_scalar(
                    out=sh_row, in0=amax_row, scalar1=EPS, scalar2=1.0 / 127.0,
                    op0=MAX, op1=MULT,
                )
                nc.vector.reciprocal(rph_row, sh_row)
                nc.vector.tensor_tensor(
                    out=rph_row, in0=rph_row, in1=sxu_row, op=MULT
                )
                nc.vector.tensor_scalar(
                    out=shd_row, in0=sh_row, scalar1=sw_cells[:, 2:3],
                    scalar2=None, op0=MULT, op1=BYP,
                )
                rph_bc = big_p.tile([P, T], F32)
                shd_bc = big_p.tile([P, T], F32)
                bcast(rph_row, rph_bc)
                bcast(shd_row, shd_bc)
                qtmp = ph2_p.tile([P, T], F32)
                for k in range(FB):
                    nc.vector.tensor_tensor(
                        out=qtmp, in0=hpT[:, k, :], in1=rph_bc, op=MULT
                    )
                    nc.vector.tensor_scalar(
                        out=hpT[:, k, :], in0=qtmp, scalar1=CR, scalar2=CR,
                        op0=ADD, op1=SUB,
                    )

                # ======== phase 3: down projection ============================
                with contextlib.ExitStack() as ph3:
                    wd3_p = ph3.enter_context(tc.tile_pool(name="wd3", bufs=3))
                    fin_p = ph3.enter_context(tc.tile_pool(name="fin", bufs=2))
                    DMQ = MQ * P
                    for q in range(MD // MQ):
                        ps3 = [
                            psum.tile([P, 512], F32, name="ps")
                            for _ in range(MQ * NTC)
                        ]
                        for g in range(FB // 4):
                            wtile = wd3_p.tile([P, 4, DMQ], FP16, name="wd3t")
                            nc.sync.dma_start(
                                wtile,
                                gout_d[
                                    g * 4 * P : (g + 1) * 4 * P,
                                    q * DMQ : (q + 1) * DMQ,
                                ].rearrange("(i p) f -> p i f", p=P),
                            )
                            for kb in range(4):
                                k = g * 4 + kb
                                st, sp = (k == 0), (k == FB - 1)
                                for mi in range(MQ):
                                    lhsT = wtile[:, kb, mi * P : (mi + 1) * P]
                                    for t in range(NTC):
                                        nc.tensor.matmul(
                                            ps3[mi * NTC + t][:, :TN],
                                            lhsT,
                                            hpT[:, k, t * TN : (t + 1) * TN],
                                            start=st,
                                            stop=sp,
                                        )
                        for mi in range(MQ):
                            md = q * MQ + mi
                            ot = fin_p.tile([P, T], F32, name="ot")
                            for t in range(NTC):
                                ts_ = slice(t * TN, (t + 1) * TN)
                                nc.vector.tensor_tensor(
                                    out=ot[:, ts_], in0=ps3[mi * NTC + t][:, :TN],
                                    in1=shd_bc[:, ts_], op=MULT,
                                )
                            nc.scalar.dma_start(
                                out_d[md * P : (md + 1) * P, :], ot
                            )

    nc.compile()
    return nc


_CACHE = {}
TRACE = False
LAST_RESULTS = None


def _get_program(T, DM, FF, ncores):
    key = (T, DM, FF, ncores)
    if key not in _CACHE:
        _CACHE[key] = build_program(T, DM, FF, ncores)
    return _CACHE[key]


def kernel(x, w_gate, w_up, w_down):
    from concourse.bass_utils import run_bass_kernel_spmd

    x = np.asarray(x, dtype=np.float32)
    w_gate = np.asarray(w_gate, dtype=np.float32)
    w_up = np.asarray(w_up, dtype=np.float32)
    w_down = np.asarray(w_down, dtype=np.float32)

    B, S, DM = x.shape
    FF = w_gate.shape[0]
    NCORES = 8
    NTOK = B * S
    T = NTOK // NCORES
    SH = FF // NCORES

    xf = np.ascontiguousarray(x.reshape(NTOK, DM))
    nc = _get_program(T, DM, FF, NCORES)

    in_maps = []
    for c in range(NCORES):
        sl = slice(c * SH, (c + 1) * SH)
        in_maps.append(
            {
                "x": np.ascontiguousarray(xf[c * T : (c + 1) * T]),
                "wgt_sh": np.ascontiguousarray(w_gate[sl].T),
                "wut_sh": np.ascontiguousarray(w_up[sl].T),
                "wdt_sh": np.ascontiguousarray(w_down[:, sl].T),
            }
        )

    res = run_bass_kernel_spmd(
        nc, in_maps, core_ids=list(range(NCORES)), trace=TRACE
    )
    global LAST_RESULTS
    LAST_RESULTS = res
    out = np.empty((NTOK, DM), dtype=np.float32)
    for c in range(NCORES):
        out[c * T : (c + 1) * T] = res.results[c]["out_t"].T
    return out.reshape(B, S, DM)


# revision 38
# speedup vs baseline: 1.0378x; 1.0368x over previous
"""BitNet FFN Trainium2 kernel (8-core SPMD).

Math (forward values of the STE reference):
  wq(w)  = clip(round(w/s), -1, 1) * s,  s = mean(|w|) + EPS        (ternary)
  xq(x)  = round(x/sx) * sx,  sx = max(absmax_row(x), EPS)/127      (int8 range)
  gate = sigmoid(xq @ wq_g.T); up = xq @ wq_u.T; h = gate*up
  out  = hq(h) @ wq_d.T

Design (v2):
  - Data-parallel over tokens (1024 tok/core), but weight ternarization is
    SHARDED: each core ternarizes 1/8 of each weight matrix (host passes the
    shard pre-transposed into matmul layout), then two AllGathers distribute
    the ternary bf16/fp16 weights to all cores.  This removes the 8x
    redundant weight DMA + tanh/round work the v1 kernel did.
  - Phase 1 computes G/U with the TERNARY WEIGHT as the stationary operand,
    so h' lands in PSUM already transposed ([ff, tok]).  h' = sigmoid(G*sxg)
    * U is written to a resident SBUF tile in fp16 -- no DRAM round trip.
  - Quantization of h' to int8-valued fp16 happens IN PLACE once the global
    per-token absmax is known; phase-3 matmuls chase the quantizer per
    k-block so the PE bubble at the boundary is tiny.
  - All matmuls run on exact integers (|int|<=127 activations, ternary
    weights, fp32 PSUM accumulation), scales are folded in fp32 outside.
    The only approximation is storing h' in fp16 before quantization.
"""

import sys

sys.path.insert(0, "/opt/trn_rl_repo")

import contextlib

import numpy as np

import concourse.tile as tile
from concourse import bacc, mybir
from concourse.masks import make_identity

F32 = mybir.dt.float32
BF16 = mybir.dt.bfloat16
FP16 = mybir.dt.float16
ADD = mybir.AluOpType.add
SUB = mybir.AluOpType.subtract
MULT = mybir.AluOpType.mult
MAX = mybir.AluOpType.max
BYP = mybir.AluOpType.bypass
AXX = mybir.AxisListType.X
AFT = mybir.ActivationFunctionType

EPS = 1e-5
CR = 12582912.0  # 1.5*2^23: fp32 RNE round-to-integer magic constant
ALPHA = 1.0986122886681098  # atanh(0.5)/0.5 : tanh(ALPHA*0.5) == 0.5
P = 128


def build_program(T, DM, FF, ncores):
    """Per-core SPMD program. T tokens/core; full DM/FF; ff sharded /ncores."""
    KD = DM // P           # d_model contraction blocks
    FB = FF // P           # ff 128-blocks
    MT = T // P            # token tiles
    SH = FF // ncores      # ff rows per shard
    SB = SH // P           # ff blocks per shard
    TN = min(512, T)       # token chunk per matmul
    NTC = T // TN          # token chunks
    CW = P                 # ff columns per phase-1 weight fetch
    MD = DM // P           # output dm blocks
    MQ = 4 if MD % 4 == 0 else 1   # dm blocks per phase-3 quad
    NW = float(FF * DM)
    assert T % P == 0 and DM % P == 0 and FF % (P * ncores) == 0
    assert FB % 4 == 0

    nc = bacc.Bacc(
        "TRN2",
        target_bir_lowering=False,
        debug=False,
        enable_asserts=False,
        num_devices=ncores,
    )

    x_d = nc.dram_tensor("x", [T, DM], F32, kind="ExternalInput")
    wgt_d = nc.dram_tensor("wgt_sh", [DM, SH], F32, kind="ExternalInput")
    wut_d = nc.dram_tensor("wut_sh", [DM, SH], F32, kind="ExternalInput")
    wdt_d = nc.dram_tensor("wdt_sh", [SH, DM], F32, kind="ExternalInput")
    out_d = nc.dram_tensor("out_t", [DM, T], F32, kind="ExternalOutput")

    RG = [list(range(ncores))]

    with tile.TileContext(nc, num_cores=ncores) as tc:
        with contextlib.ExitStack() as S:
            dram = S.enter_context(tc.tile_pool(name="dram", bufs=1, space="DRAM"))
            psum = S.enter_context(tc.tile_pool(name="psum", bufs=8, space="PSUM"))
            tiny = S.enter_context(tc.tile_pool(name="tiny", bufs=1))

            # DRAM scratch: AllGather bounce buffers.  wg/wu ternary weights
            # are gathered in NP ff-column pieces so the AllGathers pipeline
            # behind phase-1 compute.
            NP = 4 if SH % (4 * CW) == 0 else 1
            PW = SH // NP
            gin_p = [dram.tile([2 * DM, PW], BF16, name=f"ginp{i}") for i in range(NP)]
            gout_p = [
                dram.tile([ncores * 2 * DM, PW], BF16, addr_space="Shared",
                          name=f"goutp{i}")
                for i in range(NP)
            ]
            gin_d = dram.tile([SH, DM], FP16)
            gout_d = dram.tile([FF, DM], FP16, addr_space="Shared")
            ar_in = dram.tile([1, 4], F32)
            ar_out = dram.tile([1, 4], F32)
            rows_d = dram.tile([2, T], F32)   # rowify bounce (sx, amax)

            # persistent small tiles
            ones_row = tiny.tile([1, P], F32)
            nc.vector.memset(ones_row, 1.0)
            ones_col = tiny.tile([P, 1], F32)
            nc.vector.memset(ones_col, 1.0)
            ident = tiny.tile([P, P], F32)
            make_identity(nc, ident)

            sw_cells = tiny.tile([1, 3], F32)    # s_wg, s_wu, s_wd
            rsw_bc = tiny.tile([P, 3], F32)      # 1/s_w per partition
            sxu_row = tiny.tile([1, T], F32)     # sx*s_wu (survives to phase 2)
            sx_cols = tiny.tile([P, MT], F32)
            am_cols = tiny.tile([P, MT], F32)

            def rowify(cols, row, slot):
                """cols [P, MT] (col m = tokens m*P..(m+1)*P) -> row [1, T]."""
                pst = psum.tile([P, 512], F32, name="ps")
                nc.tensor.transpose(pst[:MT, :P], cols[:, :MT], ident)
                sb_t = tiny.tile([MT, P], F32, name="rowify_t")
                nc.vector.tensor_copy(sb_t, pst[:MT, :P])
                nc.sync.dma_start(rows_d[slot, :], sb_t[:, :])
                nc.sync.dma_start(row, rows_d[slot : slot + 1, :])

            def bcast(row, out_bc):
                """row [1, T] -> out_bc [P, T] (same value down partitions)."""
                for t in range(NTC):
                    psb = psum.tile([P, 512], F32, name="ps")
                    nc.tensor.matmul(
                        psb[:, :TN], ones_row, row[:, t * TN : (t + 1) * TN],
                        start=True, stop=True,
                    )
                    nc.vector.tensor_copy(out_bc[:, t * TN : (t + 1) * TN], psb[:, :TN])

            # ============ S0: sharded weight-scale scan + tiny AllReduce ======
            with contextlib.ExitStack() as pre:
                whold_p = pre.enter_context(tc.tile_pool(name="whold", bufs=1))
                wg_hold = whold_p.tile([P, KD, SH], F32)
                wu_hold = whold_p.tile([P, KD, SH], F32)
                nc.sync.dma_start(
                    wg_hold, wgt_d[:, :].rearrange("(i p) f -> p i f", p=P)
                )
                nc.scalar.dma_start(
                    wu_hold, wut_d[:, :].rearrange("(i p) f -> p i f", p=P)
                )
                acc = tiny.tile([P, 3], F32)
                nc.vector.tensor_reduce(
                    acc[:, 0:1], wg_hold.rearrange("p i f -> p (i f)"),
                    axis=AXX, op=ADD, apply_absolute_value=True,
                )
                nc.vector.tensor_reduce(
                    acc[:, 1:2], wu_hold.rearrange("p i f -> p (i f)"),
                    axis=AXX, op=ADD, apply_absolute_value=True,
                )
                # wd scan: streaming (tiles are reloaded later for ternarize)
                with tc.tile_pool(name="wdscan", bufs=3) as wds_p:
                    nc.vector.memset(acc[:, 2:3], 0.0)
                    for i in range(SB):
                        wdt = wds_p.tile([P, DM], F32, name="wdscan_t")
                        nc.gpsimd.dma_start(wdt, wdt_d[i * P : (i + 1) * P, :])
                        part = wds_p.tile([P, 1], F32, name="wdscan_s")
                        nc.vector.tensor_reduce(
                            part, wdt, axis=AXX, op=ADD, apply_absolute_value=True
                        )
                        nc.vector.tensor_tensor(
                            out=acc[:, 2:3], in0=acc[:, 2:3], in1=part, op=ADD
                        )

                pss = psum.tile([P, 512], F32, name="ps")
                nc.tensor.matmul(pss[:3, :1], acc[:, :3], ones_col, start=True, stop=True)
                sums_sb = tiny.tile([3, 1], F32)
                nc.vector.tensor_copy(sums_sb, pss[:3, :1])
                nc.scalar.dma_start(ar_in[0, :3], sums_sb[:, 0])
                nc.gpsimd.collective_compute(
                    "AllReduce",
                    ADD,
                    replica_groups=RG,
                    ins=[ar_in[:, :3].opt()],
                    outs=[ar_out[:, :3].opt()],
                )
                arld = tiny.tile([1, 3], F32)
                nc.scalar.dma_start(arld, ar_out[:, :3])
                nc.vector.tensor_scalar(
                    out=sw_cells, in0=arld, scalar1=1.0 / NW, scalar2=EPS,
                    op0=MULT, op1=ADD,
                )
                binv = tiny.tile([1, 3], F32)
                nc.vector.reciprocal(binv, sw_cells)
                psb3 = psum.tile([P, 512], F32, name="ps")
                nc.tensor.matmul(psb3[:, :3], ones_row, binv, start=True, stop=True)
                nc.vector.tensor_copy(rsw_bc, psb3[:, :3])

                # ---- ternarize wg/wu shards piece-major (DVE round+clip),
                #      kick each piece's AllGather as soon as it is bounced.
                # tq is deep-buffered so the DVE ladder is not starved by the
                # per-DMA completion latency of the bounce writes; the bounce
                # DMAs ride the scalar ring so they never block the sync-ring
                # phase-1 weight fetches.
                with tc.tile_pool(name="tern", bufs=3) as tern_p, tc.tile_pool(
                    name="ternq", bufs=2
                ) as ternq_p:
                    for ch in range(NP):
                        fs = slice(ch * PW, (ch + 1) * PW)
                        for src_hold, base, col in (
                            (wg_hold, 0, 0),
                            (wu_hold, DM, 1),
                        ):
                            # accumulate the whole [DM, PW] ternary piece in
                            # SBUF, bounce it with ONE 1MB DMA (the per-DMA
                            # ring overhead was the ladder's governor)
                            tq = ternq_p.tile([P, KD, PW], BF16, name="tern_q")
                            for i in range(KD):
                                t1 = tern_p.tile([P, PW], F32, name="tern_1")
                                nc.vector.tensor_scalar(
                                    out=t1, in0=src_hold[:, i, fs],
                                    scalar1=rsw_bc[:, col : col + 1], scalar2=1.25,
                                    op0=MULT, op1=mybir.AluOpType.min,
                                )
                                t2 = tern_p.tile([P, PW], F32, name="tern_2")
                                nc.vector.tensor_scalar(
                                    out=t2, in0=t1, scalar1=-1.25, scalar2=CR,
                                    op0=MAX, op1=ADD,
                                )
                                nc.vector.tensor_scalar(
                                    out=tq[:, i, :], in0=t2, scalar1=CR,
                                    scalar2=None, op0=SUB, op1=BYP,
                                )
                            nc.scalar.dma_start(
                                gin_p[ch][base : base + DM, :].rearrange(
                                    "(i p) f -> p i f", p=P
                                ),
                                tq,
                            )
                        nc.gpsimd.collective_compute(
                            "AllGather",
                            BYP,
                            replica_groups=RG,
                            ins=[gin_p[ch][:].opt()],
                            outs=[gout_p[ch][:].opt()],
                        )

            # ============ persistent big tiles across phases 1..3 =============
            big_p = S.enter_context(tc.tile_pool(name="big", bufs=1))
            hpT = big_p.tile([P, FB, T], FP16)    # h' (later: quantized h)
            mxa = big_p.tile([P, T], F32)         # running absmax of |h'|
            nc.vector.memset(mxa, 0.0)

            with contextlib.ExitStack() as mid:
                mid_p = mid.enter_context(tc.tile_pool(name="mid", bufs=1))
                xqt = mid_p.tile([P, KD, T], BF16)   # x quantized, transposed
                sxg_bc = mid_p.tile([P, T], FP16)

                # ======== phase 0: x quantization (overlaps the AllGather) ====
                with tc.tile_pool(name="xw", bufs=2) as xw_p:
                    for m in range(MT):
                        xt = xw_p.tile([P, DM], F32, name="xt")
                        nc.sync.dma_start(xt, x_d[m * P : (m + 1) * P, :])
                        amax = xw_p.tile([P, 1], F32, name="amax")
                        nc.vector.tensor_reduce(
                            amax, xt, axis=AXX, op=MAX, apply_absolute_value=True
                        )
                        nc.vector.tensor_scalar(
                            out=sx_cols[:, m : m + 1], in0=amax, scalar1=EPS,
                            scalar2=1.0 / 127.0, op0=MAX, op1=MULT,
                        )
                        rx = xw_p.tile([P, 1], F32, name="rx")
                        nc.vector.reciprocal(rx, sx_cols[:, m : m + 1])
                        nc.vector.tensor_scalar(
                            out=xt, in0=xt, scalar1=rx, scalar2=CR, op0=MULT, op1=ADD
                        )
                        xq = xw_p.tile([P, DM], BF16, name="xq")
                        nc.vector.tensor_scalar(
                            out=xq, in0=xt, scalar1=CR, scalar2=None, op0=SUB, op1=BYP
                        )
                        nc.sync.dma_start_transpose(
                            xqt[:, :, m * P : (m + 1) * P], xq
                        )
                with tc.tile_pool(name="ph0rows", bufs=1) as r0_p:
                    sx_row = r0_p.tile([1, T], F32, name="sx_row")
                    sxg_row = r0_p.tile([1, T], F32, name="sxg_row")
                    rowify(sx_cols, sx_row, 0)
                    nc.vector.tensor_scalar(
                        out=sxg_row, in0=sx_row, scalar1=sw_cells[:, 0:1],
                        scalar2=None, op0=MULT, op1=BYP,
                    )
                    nc.vector.tensor_scalar(
                        out=sxu_row, in0=sx_row, scalar1=sw_cells[:, 1:2],
                        scalar2=None, op0=MULT, op1=BYP,
                    )
                    bcast(sxg_row, sxg_bc)

                def ternarize_wd(td_p):
                    """wd shard ternarize entirely on GpSimd (loads + math +
                    stores) -> gin_d -> AG.  Runs mid-phase-1: steals no DVE
                    time and its AG lands well before phase 3 needs it."""
                    W3 = min(1024, DM)
                    for i in range(SB):
                        for c0 in range(0, DM, W3):
                            wdt = td_p.tile([P, W3], F32, name="ternd_in")
                            nc.gpsimd.dma_start(
                                wdt, wdt_d[i * P : (i + 1) * P, c0 : c0 + W3]
                            )
                            nc.gpsimd.tensor_scalar(
                                out=wdt, in0=wdt, scalar1=rsw_bc[:, 2:3],
                                scalar2=1.25, op0=MULT, op1=mybir.AluOpType.min,
                            )
                            nc.gpsimd.tensor_scalar(
                                out=wdt, in0=wdt, scalar1=-1.25, scalar2=CR,
                                op0=MAX, op1=ADD,
                            )
                            tqd = td_p.tile([P, W3], FP16, name="ternd_q")
                            nc.gpsimd.tensor_scalar(
                                out=tqd, in0=wdt, scalar1=CR, scalar2=None,
                                op0=SUB, op1=BYP,
                            )
                            nc.gpsimd.dma_start(
                                gin_d[i * P : (i + 1) * P, c0 : c0 + W3], tqd
                            )
                    nc.gpsimd.collective_compute(
                        "AllGather",
                        BYP,
                        replica_groups=RG,
                        ins=[gin_d[:].opt()],
                        outs=[gout_d[:].opt()],
                    )

                # ======== phase 1: gate/up matmuls, h' -> hpT (fp16) ==========
                # piece-outer so each block only waits on its piece's AG
                with contextlib.ExitStack() as ph1:
                    wch_p = ph1.enter_context(tc.tile_pool(name="wch", bufs=2))
                    gt_p = ph1.enter_context(tc.tile_pool(name="gt", bufs=1))
                    td_p = ph1.enter_context(tc.tile_pool(name="ternd", bufs=1))
                    for ch in range(NP):
                        if ch == min(2, NP - 1):
                            ternarize_wd(td_p)
                        for c in range(ncores):
                            for sub in range(PW // CW):
                                # one fetch holds the G and U k-stacks for
                                # this 128-wide ff block: [P, 2*KD, CW]
                                guch = wch_p.tile(
                                    [P, 2 * KD, CW], BF16, name="guch"
                                )
                                base = c * 2 * DM
                                cs = slice(sub * CW, (sub + 1) * CW)
                                nc.sync.dma_start(
                                    guch,
                                    gout_p[ch][
                                        base : base + 2 * DM, cs
                                    ].rearrange("(i p) f -> p i f", p=P),
                                )
                                f = c * SB + ch * (PW // P) + sub
                                psG = [
                                    psum.tile([P, 512], F32, name="ps")
                                    for _ in range(NTC)
                                ]
                                psU = [
                                    psum.tile([P, 512], F32, name="ps")
                                    for _ in range(NTC)
                                ]
                                for ps_list, koff in ((psG, 0), (psU, KD)):
                                    for k in range(KD):
                                        lhsT = guch[:, koff + k, :]
                                        st, sp = (k == 0), (k == KD - 1)
                                        for t in range(NTC):
                                            nc.tensor.matmul(
                                                ps_list[t][:, :TN],
                                                lhsT,
                                                xqt[:, k, t * TN : (t + 1) * TN],
                                                start=st,
                                                stop=sp,
                                            )
                                gt = gt_p.tile([P, T], F32, name="gt")
                                for t in range(NTC):
                                    ts_ = slice(t * TN, (t + 1) * TN)
                                    nc.vector.tensor_tensor(
                                        out=gt[:, ts_], in0=psG[t][:, :TN],
                                        in1=sxg_bc[:, ts_], op=MULT,
                                    )
                                nc.scalar.activation(
                                    out=gt, in_=gt, func=AFT.Sigmoid
                                )
                                for t in range(NTC):
                                    ts_ = slice(t * TN, (t + 1) * TN)
                                    nc.vector.tensor_tensor(
                                        out=hpT[:, f, ts_], in0=gt[:, ts_],
                                        in1=psU[t][:, :TN], op=MULT,
                                    )
                                # gt is dead after the h' write: reuse it as
                                # the |h'| scratch for the running absmax
                                nc.scalar.activation(
                                    out=gt, in_=hpT[:, f, :], func=AFT.Abs
                                )
                                nc.vector.tensor_tensor(
                                    out=mxa, in0=mxa, in1=gt, op=MAX
                                )

            # ============ phase 2: global h scales + in-place quantize ========
            with contextlib.ExitStack() as ph2:
                ph2_p = ph2.enter_context(tc.tile_pool(name="ph2", bufs=1))
                # per-token absmax over partitions: PE transpose + free-axis max
                for m in range(MT):
                    pst = psum.tile([P, 512], F32, name="ps")
                    nc.tensor.transpose(
                        pst[:P, :P], mxa[:, m * P : (m + 1) * P], ident
                    )
                    nc.vector.tensor_reduce(
                        am_cols[:, m : m + 1], pst[:P, :P], axis=AXX, op=MAX
                    )
                amax_row = ph2_p.tile([1, T], F32)
                sh_row = ph2_p.tile([1, T], F32)
                rph_row = ph2_p.tile([1, T], F32)
                shd_row = ph2_p.tile([1, T], F32)
                rowify(am_cols, amax_row, 1)
                nc.vector.tensor_tensor(
                    out=amax_row, in0=amax_row, in1=sxu_row, op=MULT
                )
                nc.vector.tensor_scalar(
                    out=sh_row, in0=amax_row, scalar1=EPS, scalar2=1.0 / 127.0,
                    op0=MAX, op1=MULT,
                )
                nc.vector.reciprocal(rph_row, sh_row)
                nc.vector.tensor_tensor(
                    out=rph_row, in0=rph_row, in1=sxu_row, op=MULT
                )
                nc.vector.tensor_scalar(
                    out=shd_row, in0=sh_row, scalar1=sw_cells[:, 2:3],
                    scalar2=None, op0=MULT, op1=BYP,
                )
                rph_bc = big_p.tile([P, T], F32)
                shd_bc = big_p.tile([P, T], F32)
                bcast(rph_row, rph_bc)
                bcast(shd_row, shd_bc)
                qtmp = ph2_p.tile([P, T], F32)
                for k in range(FB):
                    nc.vector.tensor_tensor(
                        out=qtmp, in0=hpT[:, k, :], in1=rph_bc, op=MULT
                    )
                    nc.vector.tensor_scalar(
                        out=hpT[:, k, :], in0=qtmp, scalar1=CR, scalar2=CR,
                        op0=ADD, op1=SUB,
                    )

                # ======== phase 3: down projection ============================
                with contextlib.ExitStack() as ph3:
                    wd3_p = ph3.enter_context(tc.tile_pool(name="wd3", bufs=3))
                    fin_p = ph3.enter_context(tc.tile_pool(name="fin", bufs=2))
                    DMQ = MQ * P
                    for q in range(MD // MQ):
                        ps3 = [
                            psum.tile([P, 512], F32, name="ps")
                            for _ in range(MQ * NTC)
                        ]
                        for g in range(FB // 4):
                            wtile = wd3_p.tile([P, 4, DMQ], FP16, name="wd3t")
                            nc.sync.dma_start(
                                wtile,
                                gout_d[
                                    g * 4 * P : (g + 1) * 4 * P,
                                    q * DMQ : (q + 1) * DMQ,
                                ].rearrange("(i p) f -> p i f", p=P),
                            )
                            for kb in range(4):
                                k = g * 4 + kb
                                st, sp = (k == 0), (k == FB - 1)
                                for mi in range(MQ):
                                    lhsT = wtile[:, kb, mi * P : (mi + 1) * P]
                                    for t in range(NTC):
                                        nc.tensor.matmul(
                                            ps3[mi * NTC + t][:, :TN],
                                            lhsT,
                                            hpT[:, k, t * TN : (t + 1) * TN],
                                            start=st,
                                            stop=sp,
                                        )
                        for mi in range(MQ):
                            md = q * MQ + mi
                            ot = fin_p.tile([P, T], F32, name="ot")
                            for t in range(NTC):
                                ts_ = slice(t * TN, (t + 1) * TN)
                                nc.vector.tensor_tensor(
                                    out=ot[:, ts_], in0=ps3[mi * NTC + t][:, :TN],
                                    in1=shd_bc[:, ts_], op=MULT,
                                )
                            nc.scalar.dma_start(
                                out_d[md * P : (md + 1) * P, :], ot
                            )

    nc.compile()
    return nc


_CACHE = {}
TRACE = False
LAST_RESULTS = None


def _get_program(T, DM, FF, ncores):
    key = (T, DM, FF, ncores)
    if key not in _CACHE:
        _CACHE[key] = build_program(T, DM, FF, ncores)
    return _CACHE[key]


def kernel(x, w_gate, w_up, w_down):
    from concourse.bass_utils import run_bass_kernel_spmd

    x = np.asarray(x, dtype=np.float32)
    w_gate = np.asarray(w_gate, dtype=np.float32)
    w_up = np.asarray(w_up, dtype=np.float32)
    w_down = np.asarray(w_down, dtype=np.float32)

    B, S, DM = x.shape
    FF = w_gate.shape[0]
    NCORES = 8
    NTOK = B * S
    T = NTOK // NCORES
    SH = FF // NCORES

    xf = np.ascontiguousarray(x.reshape(NTOK, DM))
    nc = _get_program(T, DM, FF, NCORES)

    in_maps = []
    for c in range(NCORES):
        sl = slice(c * SH, (c + 1) * SH)
        in_maps.append(
            {
                "x": np.ascontiguousarray(xf[c * T : (c + 1) * T]),
                "wgt_sh": np.ascontiguousarray(w_gate[sl].T),
                "wut_sh": np.ascontiguousarray(w_up[sl].T),
                "wdt_sh": np.ascontiguousarray(w_down[:, sl].T),
            }
        )

    res = run_bass_kernel_spmd(
        nc, in_maps, core_ids=list(range(NCORES)), trace=TRACE
    )
    global LAST_RESULTS
    LAST_RESULTS = res
    out = np.empty((NTOK, DM), dtype=np.float32)
    for c in range(NCORES):
        out[c * T : (c + 1) * T] = res.results[c]["out_t"].T
    return out.reshape(B, S, DM)
